# revision 76
# baseline (speedup 1.0000x reference)
"""Multi-head self-attention (B=4, S=2048, D=1024, H=16, RoPE, causal) on 8 trn2 cores.

Sharding: core c -> batch c//2, heads [8*(c%2), 8*(c%2)+8)   (2 cores per batch,
each doing 8 of the 16 heads).  Each core computes its partial output
projection out^T [1024, 2048]; host sums the two halves per batch and
transposes back.

All matmuls bf16 (f32 psum).  Layout is transposed throughout: x^T [D,S] in
SBUF (resident), Q^T/K^T [dk,s], scores^T [k,q] (softmax sum via a
ones-column appended to V in the attn@V matmul), out^T [o,s].

Per kc, the two heads' scores land in one [128, h0|h1] psum tile: the two
K=64 matmuls run CONCURRENTLY in PE row groups (0,0)/(64,0), and one exp
covers both heads.  Causal masking: diagonal matmuls restrict their column
range (fully-masked part skipped) and a [128,128] triangular 0/1 multiply
zeroes the staircase.  proj/vt of head-pair hp+1 are software-pipelined into
the (scalar-bound) attention of hp; outproj is interleaved into attn3 per
q-chunk after per-chunk normalization.
"""
import sys
sys.path.insert(0, "/opt/trn_rl_repo")
import math
from contextlib import ExitStack
import numpy as np
import ml_dtypes

import concourse.bass as bass
import concourse.bacc as bacc
import concourse.mybir as mybir
from concourse.tile import TileContext
from concourse.bass_utils import run_bass_kernel_spmd

F32 = mybir.dt.float32
BF16 = mybir.dt.bfloat16
ADT = BF16

B, S, D, H, DK = 4, 2048, 1024, 16, 64
NCORES = 8
NPAIR = 4               # head pairs per core
QC = 512                # q chunk (matmul moving free size)
NQC = S // QC           # 4
KC = 128                # k chunk (scores psum partition dim)
NKC = S // KC           # 16
SC = 512                # s chunk for projections / outproj
NSC = S // SC           # 4

_BUILT = {}


def _build_nc():
    nc = bacc.Bacc()

    xT_d = nc.declare_dram_parameter("xT", [2, 128, 8, S // 2], BF16, isOutput=False)
    wqkv_d = nc.declare_dram_parameter("wqkv", [NPAIR, 128, 3, D], BF16, isOutput=False)
    wo_d = nc.declare_dram_parameter("wopk", [128, 8, NPAIR, 128], BF16, isOutput=False)
    ctab_d = nc.declare_dram_parameter("ctab", [128, S], BF16, isOutput=False)
    stab_d = nc.declare_dram_parameter("stab", [128, S], BF16, isOutput=False)
    mk_d = nc.declare_dram_parameter("masks", [KC, KC], ADT, isOutput=False)
    id_d = nc.declare_dram_parameter("ident", [128, 128], ADT, isOutput=False)
    idT_d = nc.declare_dram_parameter("identT", [128, 64], F32, isOutput=False)
    out_d = nc.declare_dram_parameter("outP", [D, S], F32, isOutput=True)

    swapmask = [i ^ 1 for i in range(32)]

    with TileContext(nc) as tc, ExitStack() as ctx:
        ep = ctx.enter_context
        consts = ep(tc.tile_pool(name="consts", bufs=1))
        xpool = ep(tc.tile_pool(name="xt", bufs=2))
        wpool = ep(tc.tile_pool(name="w", bufs=4))
        wopool = ep(tc.tile_pool(name="wo", bufs=1))
        vpool = ep(tc.tile_pool(name="vraw", bufs=2))
        rupool = ep(tc.tile_pool(name="ru", bufs=4))
        rvpool = ep(tc.tile_pool(name="rv", bufs=4))
        qkpool = ep(tc.tile_pool(name="qk", bufs=4))
        v1pool = ep(tc.tile_pool(name="v1", bufs=4))
        ppool = ep(tc.tile_pool(name="pT", bufs=6))
        orawpool = ep(tc.tile_pool(name="oraw", bufs=2))
        sumpool = ep(tc.tile_pool(name="sums", bufs=4))
        recpool = ep(tc.tile_pool(name="rec", bufs=4))
        otpool = ep(tc.tile_pool(name="oT", bufs=4))
        obpool = ep(tc.tile_pool(name="ob", bufs=4))
        drpool = ep(tc.tile_pool(name="dr", bufs=4, space="DRAM"))
        psA = ep(tc.tile_pool(name="psA", bufs=2, space="PSUM"))
        psB = ep(tc.tile_pool(name="psB", bufs=2, space="PSUM"))
        psO = ep(tc.tile_pool(name="psO", bufs=2, space="PSUM"))

        # ---- startup DMAs; x arrives per (ic, sc) chunk so the first
        # projection group is compute-ready at ~3us ----
        # small consts ride the Act ring first; x comes as two single 2MB
        # DMAs on the SP ring
        tri = consts.tile([KC, KC], ADT)
        ident = consts.tile([128, 128], ADT)
        identT = consts.tile([128, 64], F32)
        nc.scalar.dma_start(out=tri, in_=mk_d[:, :])
        nc.scalar.dma_start(out=ident, in_=id_d[:, :])
        nc.scalar.dma_start(out=identT, in_=idT_d[:, :])

        xres = [[None] * NSC for _ in range(8)]
        ctabs, stabs = [None] * NSC, [None] * NSC

        def _ldct(sc):
            ct = consts.tile([128, SC], BF16, tag=f"ct{sc}", name=f"ct{sc}")
            st = consts.tile([128, SC], BF16, tag=f"st{sc}", name=f"st{sc}")
            nc.scalar.dma_start(out=ct, in_=ctab_d[:, sc * SC:(sc + 1) * SC])
            nc.scalar.dma_start(out=st, in_=stab_d[:, sc * SC:(sc + 1) * SC])
            ctabs[sc] = ct
            stabs[sc] = st

        for half in range(2):
            xv = xpool.tile([128, 8, S // 2], BF16, tag="xt",
                            name=f"xv{half}")
            nc.sync.dma_start(out=xv, in_=xT_d[half])
            for ic in range(8):
                xres[ic][2 * half] = xv[:, ic, 0:SC]
                xres[ic][2 * half + 1] = xv[:, ic, SC:2 * SC]
            if half == 0:
                wqkvs = []
                w0 = wpool.tile([128, 3, D], BF16, tag="w", name="wqkv0")
                nc.sync.dma_start(out=w0, in_=wqkv_d[0])
                wqkvs.append(w0)
                _ldct(0)
                _ldct(1)
            else:
                _ldct(2)
                _ldct(3)

        for hp in range(1, NPAIR):
            w = wpool.tile([128, 3, D], BF16, tag="w", name=f"wqkv{hp}")
            nc.sync.dma_start(out=w, in_=wqkv_d[hp])
            wqkvs.append(w)
        wot = wopool.tile([128, 8, NPAIR, 128], BF16)
        nc.sync.dma_start(out=wot, in_=wo_d[:, :, :, :])

        state = {}   # hp -> (qT, kT, v1s)
        oTs = []

        def proj_steps(hp):
            """12 proj psum-group closures + 2 V-transpose closures."""
            qT = qkpool.tile([128, S], ADT, tag="qk", name=f"qT{hp}")
            kT = qkpool.tile([128, S], ADT, tag="qk", name=f"kT{hp}")
            vraw = vpool.tile([128, S], F32, tag="vraw", name=f"vraw{hp}")
            v1s = []
            state[hp] = (qT, kT, v1s)
            steps = []

            def pstep(sc, pj):
                with nc.named_scope(f"proj{hp}"):
                    ssl = slice(sc * SC, (sc + 1) * SC)
                    ps = psA.tile([128, SC], F32, tag="psA",
                                  name=f"pp{hp}_{sc}_{pj}")
                    for ic in range(8):
                        nc.tensor.matmul(
                            ps, wqkvs[hp][:, pj, ic * 128:(ic + 1) * 128],
                            xres[ic][sc],
                            start=(ic == 0), stop=(ic == 7))
                    if pj < 2:  # Q or K: RoPE; psum staged to bf16 on the
                        # scalar engine so the DVE ops run at 16-bit 2x rate
                        dst = qT if pj == 0 else kT
                        qr = rvpool.tile([128, SC], ADT, tag="qr",
                                         name=f"qr{hp}_{sc}_{pj}")
                        nc.scalar.copy(out=qr, in_=ps)
                        sh = rupool.tile([128, SC], ADT, tag="ru",
                                         name=f"sh{hp}_{sc}_{pj}")
                        nc.vector.stream_shuffle(out=sh, in_=qr, mask=swapmask)
                        t1 = rvpool.tile([128, SC], ADT, tag="rv",
                                         name=f"t1{hp}_{sc}_{pj}")
                        nc.vector.tensor_mul(out=t1, in0=qr, in1=ctabs[sc])
                        t2 = rupool.tile([128, SC], ADT, tag="ru",
                                         name=f"t2{hp}_{sc}_{pj}")
                        nc.vector.tensor_mul(out=t2, in0=sh, in1=stabs[sc])
                        nc.vector.tensor_add(out=dst[:, ssl], in0=t1, in1=t2)
                    else:       # V: drain to SBUF for PE transpose
                        nc.scalar.copy(out=vraw[:, ssl], in_=ps)

            v1boxes = [None, None]

            def vtstep(h, half):
                with nc.named_scope(f"vt{hp}"):
                    if v1boxes[h] is None:
                        v1boxes[h] = v1pool.tile([128, NKC, 65], ADT, tag="v1",
                                                 name=f"v1_{hp}_{h}")
                        v1s.append(v1boxes[h])
                        nc.vector.memset(v1boxes[h][:, :, 64:65], 1.0)
                    v1 = v1boxes[h]
                    pvt = psA.tile([128, 512], F32, tag="psA",
                                   name=f"pvt{hp}_{h}_{half}")
                    for j in range(8):
                        kc = half * 8 + j
                        nc.tensor.transpose(
                            pvt[:, j * 64:(j + 1) * 64],
                            vraw[h * 64:(h + 1) * 64, kc * 128:(kc + 1) * 128],
                            identT[h * 64:(h + 1) * 64, 0:64])
                    nc.vector.tensor_copy(
                        out=v1[:, half * 8:(half + 1) * 8, 0:64],
                        in_=pvt.rearrange("p (kc d) -> p kc d", d=64))

            # q/k (and their rope) first — attention of hp+1 needs qT/kT
            # and the first v1 keys earliest
            for sc in range(NSC):
                steps.append(lambda sc=sc: pstep(sc, 0))
                steps.append(lambda sc=sc: pstep(sc, 1))
                steps.append(lambda sc=sc: pstep(sc, 2))
                if sc == 1:
                    steps.append(lambda: vtstep(0, 0))
                    steps.append(lambda: vtstep(1, 0))
            steps.append(lambda: vtstep(0, 1))
            steps.append(lambda: vtstep(1, 1))
            return steps

        def attention(hp, pending):
            """attn for hp; runs `pending` closures spread over the qc loop;
            for hp==3 interleaves the output projection per q-chunk."""
            qT, kT, v1s = state[hp]
            _sc = nc.named_scope(f"attn{hp}"); _sc.__enter__()
            oT = otpool.tile([128, S], ADT, tag="oT", name=f"oT{hp}")
            oraw = orawpool.tile([128, S], F32, tag="oraw", name=f"oraw{hp}")
            hsl = (slice(0, 64), slice(64, 128))
            nsteps = len(pending)
            nkc_tot = sum(4 * q + 4 for q in range(NQC))
            kc_seen = 0
            for qc in range(NQC):
                nact = 4 * qc + 4
                qsl = slice(qc * QC, (qc + 1) * QC)
                pquads = []
                for kc in range(nact):
                    m = kc - 4 * qc
                    lo = 128 * m if m > 0 else 0
                    psq = psB.tile([128, 1024], F32, tag="big",
                                   name=f"psq{hp}_{qc}_{kc}")
                    for h in range(2):
                        nc.tensor.matmul(
                            psq[:, h * QC + lo:(h + 1) * QC],
                            kT[hsl[h], kc * KC:(kc + 1) * KC],
                            qT[hsl[h], qc * QC + lo:(qc + 1) * QC],
                            start=True, stop=(m < 0),
                            skip_group_check=True)
                    if m >= 0:
                        # staircase: add -1e9 upper-triangle on the 128-col
                        # diagonal block (keeps exp->attnV off the DVE path)
                        for h in range(2):
                            a = h * QC + lo
                            nc.tensor.matmul(
                                psq[:, a:a + 128], ident, tri,
                                start=False, stop=True,
                                skip_group_check=True)
                    pq = ppool.tile([128, 1024], ADT, tag="pT",
                                    name=f"pq{hp}_{qc}_{kc}")
                    nc.scalar.activation(
                        out=pq, in_=psq,
                        func=mybir.ActivationFunctionType.Exp, scale=0.125)
                    pquads.append(pq)
                    # spread the next head-pair's proj/vt work evenly
                    kc_seen += 1
                    want = (nsteps * kc_seen) // nkc_tot
                    while pending and nsteps - len(pending) < want:
                        pending.pop(0)()
                psos = [psO.tile([65, QC], F32, tag="psO",
                                 name=f"pso{hp}_{qc}_{h}") for h in range(2)]
                for kc in range(nact):
                    m = kc - 4 * qc
                    lo = 128 * m if m > 0 else 0
                    for h in range(2):
                        nc.tensor.matmul(
                            psos[h][:, lo:QC], v1s[h][:, kc, :],
                            pquads[kc][:, h * QC + lo:(h + 1) * QC],
                            start=(kc == 0), stop=(kc == nact - 1),
                            skip_group_check=True)
                # normalization for this q-chunk (both heads); the 1/sum
                # partition-broadcast bounces through DRAM, keeping the long
                # dependency chain off the in-order compute engines
                sq = sumpool.tile([33, QC], F32, tag="sums",
                                  name=f"sq{hp}_{qc}")
                rq = recpool.tile([33, QC], F32, tag="rec",
                                  name=f"rq{hp}_{qc}")
                drt = drpool.tile([2, QC], F32, name=f"drt{hp}_{qc}")
                recB = recpool.tile([128, QC], F32, tag="recB",
                                    name=f"recB{hp}_{qc}")
                for h in range(2):
                    nc.vector.tensor_copy(out=oraw[hsl[h], qsl],
                                          in_=psos[h][0:64, :])
                    nc.vector.tensor_copy(out=sq[32 * h:32 * h + 1, :],
                                          in_=psos[h][64:65, :])
                    nc.vector.reciprocal(
                        out=rq[32 * h:32 * h + 1, :],
                        in_=sq[32 * h:32 * h + 1, :])
                    nc.sync.dma_start(out=drt[h:h + 1, :],
                                      in_=rq[32 * h:32 * h + 1, :])
                    nc.sync.dma_start(
                        out=recB[hsl[h], :],
                        in_=drt[h:h + 1, :].to_broadcast((64, QC)))
                nc.vector.tensor_mul(out=oT[:, qsl], in0=oraw[:, qsl],
                                     in1=recB)
                if hp == NPAIR - 1:
                    with nc.named_scope("outproj"):
                        for oc in range(8):
                            ps = psA.tile([128, SC], F32, tag="psA",
                                          name=f"ops{oc}_{qc}")
                            for h2 in range(NPAIR):
                                src = oTs[h2] if h2 < len(oTs) else oT
                                nc.tensor.matmul(
                                    ps, wot[:, oc, h2, :], src[:, qsl],
                                    start=(h2 == 0), stop=(h2 == NPAIR - 1))
                            ob = obpool.tile([128, SC], F32, tag="ob",
                                             name=f"ob{oc}_{qc}")
                            nc.vector.tensor_copy(out=ob, in_=ps)
                            nc.sync.dma_start(
                                out=out_d[oc * 128:(oc + 1) * 128, qsl],
                                in_=ob)
            while pending:
                pending.pop(0)()
            _sc.__exit__(None, None, None)
            oTs.append(oT)

        # serial phases: proj/vt{hp} fully before attn{hp} (interleave via
        # INTERLEAVE=True is experimental)
        steps0 = proj_steps(0)
        for st_ in steps0[:10]:
            st_()
        for hp in range(NPAIR):
            pending = steps0[10:] if hp == 0 else []
            if hp + 1 < NPAIR:
                pending = pending + proj_steps(hp + 1)
            attention(hp, pending)

    nc.compile()
    return nc


def get_nc():
    if "nc" not in _BUILT:
        _BUILT["nc"] = _build_nc()
    return _BUILT["nc"]


def _host_prep(x, Wq, Wk, Wv, Wo, token_positions):
    bf = ml_dtypes.bfloat16
    pos = np.asarray(token_positions).astype(np.float32)
    half = DK // 2
    inv_freq = 1.0 / (10000.0 ** (np.arange(half, dtype=np.float32) * 2.0 / DK))
    ang = pos[:, None] * inv_freq[None, :]          # [S, 32]
    cos = np.cos(ang).astype(np.float32)            # [S, 32]
    sin = np.sin(ang).astype(np.float32)
    p = np.arange(128)
    j = (p % 64) // 2
    sign = np.where(p % 2 == 0, -1.0, 1.0).astype(np.float32)
    ctab = np.ascontiguousarray(cos[:, j].T).astype(bf)                 # [128, S]
    stab = np.ascontiguousarray(sin[:, j].T * sign[:, None]).astype(bf)

    kk = np.arange(KC)[:, None]
    cc = np.arange(KC)[None, :]
    tri = np.where(cc >= kk, 0.0, -1e9).astype(bf)   # [KC, KC] additive mask
    ident = np.eye(128, dtype=bf)
    identT = np.vstack([np.eye(64, dtype=np.float32)] * 2)

    in_maps = []
    for c in range(NCORES):
        b, hf = divmod(c, 2)
        m = {}
        # [2, 128, 8, S/2]: xpk[half, p, ic, s] = xT[ic*128+p, half*S/2+s]
        m["xT"] = np.ascontiguousarray(
            x[b].T.reshape(8, 128, 2, S // 2).transpose(2, 1, 0, 3)
        ).astype(bf)
        # wqkv[hp, p, pj, ic*128+j] = Wpj[hf*512 + hp*128 + j, ic*128 + p]
        wqkv = np.empty((NPAIR, 128, 3, D), dtype=bf)
        for pj, W in enumerate((Wq, Wk, Wv)):
            Ws = W[hf * 512:(hf + 1) * 512, :]       # [512 out, 1024 in]
            A = Ws.reshape(NPAIR, 128, 8, 128)       # [hp, jout, ic, pin]
            wqkv[:, :, pj, :] = A.transpose(0, 3, 2, 1).reshape(NPAIR, 128, D)
        m["wqkv"] = wqkv
        # wopk[p, oc, hp, j] = WoT[hp*128+p, oc*128+j];  WoT = Wo[:, cols].T
        WoT = Wo[:, hf * 512:(hf + 1) * 512].T       # [512, 1024]
        Bm = WoT.reshape(NPAIR, 128, 8, 128)         # [hp, p, oc, j]
        m["wopk"] = np.ascontiguousarray(
            Bm.transpose(1, 2, 0, 3)).astype(bf)     # [128, 8, NPAIR, 128]
        m["ctab"] = ctab
        m["stab"] = stab
        m["masks"] = tri
        m["ident"] = ident
        m["identT"] = identT
        in_maps.append(m)
    return in_maps


def run(inputs, trace=False, **kw):
    in_maps = _host_prep(**{k: np.asarray(v) for k, v in inputs.items()})
    nc = get_nc()
    res = run_bass_kernel_spmd(nc, in_maps, list(range(NCORES)), trace=trace, **kw)
    outs = [res.results[c]["outP"] for c in range(NCORES)]
    out = np.stack([(outs[2 * b] + outs[2 * b + 1]).T for b in range(B)])
    return out.astype(np.float32), res


def kernel(**inputs):
    out, _ = run(inputs, trace=False)
    return out


# revision 77
# speedup vs baseline: 1.0024x; 1.0024x over previous
"""Multi-head self-attention (B=4, S=2048, D=1024, H=16, RoPE, causal) on 8 trn2 cores.

Sharding: core c -> batch c//2, heads [8*(c%2), 8*(c%2)+8)   (2 cores per batch,
each doing 8 of the 16 heads).  Each core computes its partial output
projection out^T [1024, 2048]; host sums the two halves per batch and
transposes back.

All matmuls bf16 (f32 psum).  Layout is transposed throughout: x^T [D,S] in
SBUF (resident), Q^T/K^T [dk,s], scores^T [k,q] (softmax sum via a
ones-column appended to V in the attn@V matmul), out^T [o,s].

Per kc, the two heads' scores land in one [128, h0|h1] psum tile: the two
K=64 matmuls run CONCURRENTLY in PE row groups (0,0)/(64,0), and one exp
covers both heads.  Causal masking: diagonal matmuls restrict their column
range (fully-masked part skipped) and a trimmed [128,128] additive -1e9
triangle matmul handles the staircase in psum (keeping the exp->attnV path
off the DVE).  RoPE is staged psum->bf16 on the scalar engine so its DVE ops
run at 16-bit 2x rate.  proj/vt of head-pair hp+1 are software-pipelined
into the (scalar-bound) attention of hp; outproj is interleaved into attn3
per q-chunk after per-chunk normalization (1/sum broadcast via a DRAM-bounce
DMA, which keeps that long dependency chain off the in-order engines).
"""
import sys
sys.path.insert(0, "/opt/trn_rl_repo")
import math
from contextlib import ExitStack
import numpy as np
import ml_dtypes

import concourse.bass as bass
import concourse.bacc as bacc
import concourse.mybir as mybir
from concourse.tile import TileContext
from concourse.bass_utils import run_bass_kernel_spmd

F32 = mybir.dt.float32
BF16 = mybir.dt.bfloat16
ADT = BF16

B, S, D, H, DK = 4, 2048, 1024, 16, 64
NCORES = 8
NPAIR = 4               # head pairs per core
QC = 512                # q chunk (matmul moving free size)
NQC = S // QC           # 4
KC = 128                # k chunk (scores psum partition dim)
NKC = S // KC           # 16
SC = 512                # s chunk for projections / outproj
NSC = S // SC           # 4

_BUILT = {}


def _build_nc():
    nc = bacc.Bacc()

    xT_d = nc.declare_dram_parameter("xT", [2, 128, 8, S // 2], BF16, isOutput=False)
    wqkv_d = nc.declare_dram_parameter("wqkv", [NPAIR, 128, 3, D], BF16, isOutput=False)
    wo_d = nc.declare_dram_parameter("wopk", [128, 8, NPAIR, 128], BF16, isOutput=False)
    ctab_d = nc.declare_dram_parameter("ctab", [128, S], BF16, isOutput=False)
    stab_d = nc.declare_dram_parameter("stab", [128, S], BF16, isOutput=False)
    mk_d = nc.declare_dram_parameter("masks", [KC, KC], ADT, isOutput=False)
    id_d = nc.declare_dram_parameter("ident", [128, 128], ADT, isOutput=False)
    idT_d = nc.declare_dram_parameter("identT", [128, 64], F32, isOutput=False)
    out_d = nc.declare_dram_parameter("outP", [D, S], F32, isOutput=True)

    swapmask = [i ^ 1 for i in range(32)]

    with TileContext(nc) as tc, ExitStack() as ctx:
        ep = ctx.enter_context
        consts = ep(tc.tile_pool(name="consts", bufs=1))
        xpool = ep(tc.tile_pool(name="xt", bufs=2))
        wpool = ep(tc.tile_pool(name="w", bufs=4))
        wopool = ep(tc.tile_pool(name="wo", bufs=1))
        vpool = ep(tc.tile_pool(name="vraw", bufs=2))
        rupool = ep(tc.tile_pool(name="ru", bufs=4))
        rvpool = ep(tc.tile_pool(name="rv", bufs=4))
        qkpool = ep(tc.tile_pool(name="qk", bufs=4))
        v1pool = ep(tc.tile_pool(name="v1", bufs=4))
        ppool = ep(tc.tile_pool(name="pT", bufs=6))
        orawpool = ep(tc.tile_pool(name="oraw", bufs=2))
        sumpool = ep(tc.tile_pool(name="sums", bufs=4))
        recpool = ep(tc.tile_pool(name="rec", bufs=4))
        otpool = ep(tc.tile_pool(name="oT", bufs=4))
        obpool = ep(tc.tile_pool(name="ob", bufs=4))
        drpool = ep(tc.tile_pool(name="dr", bufs=4, space="DRAM"))
        psA = ep(tc.tile_pool(name="psA", bufs=2, space="PSUM"))
        psB = ep(tc.tile_pool(name="psB", bufs=2, space="PSUM"))
        psO = ep(tc.tile_pool(name="psO", bufs=2, space="PSUM"))

        # ---- startup DMAs; x arrives per (ic, sc) chunk so the first
        # projection group is compute-ready at ~3us ----
        # small consts ride the Act ring first; x comes as two single 2MB
        # DMAs on the SP ring
        tri = consts.tile([KC, KC], ADT)
        ident = consts.tile([128, 128], ADT)
        identT = consts.tile([128, 64], F32)
        nc.scalar.dma_start(out=tri, in_=mk_d[:, :])
        nc.scalar.dma_start(out=ident, in_=id_d[:, :])
        nc.scalar.dma_start(out=identT, in_=idT_d[:, :])

        xres = [[None] * NSC for _ in range(8)]
        ctabs, stabs = [None] * NSC, [None] * NSC

        def _ldct(sc):
            ct = consts.tile([128, SC], BF16, tag=f"ct{sc}", name=f"ct{sc}")
            st = consts.tile([128, SC], BF16, tag=f"st{sc}", name=f"st{sc}")
            nc.scalar.dma_start(out=ct, in_=ctab_d[:, sc * SC:(sc + 1) * SC])
            nc.scalar.dma_start(out=st, in_=stab_d[:, sc * SC:(sc + 1) * SC])
            ctabs[sc] = ct
            stabs[sc] = st

        for half in range(2):
            xv = xpool.tile([128, 8, S // 2], BF16, tag="xt",
                            name=f"xv{half}")
            nc.sync.dma_start(out=xv, in_=xT_d[half])
            for ic in range(8):
                xres[ic][2 * half] = xv[:, ic, 0:SC]
                xres[ic][2 * half + 1] = xv[:, ic, SC:2 * SC]
            if half == 0:
                wqkvs = []
                w0 = wpool.tile([128, 3, D], BF16, tag="w", name="wqkv0")
                nc.sync.dma_start(out=w0, in_=wqkv_d[0])
                wqkvs.append(w0)
                _ldct(0)
                _ldct(1)
            else:
                _ldct(2)
                _ldct(3)

        for hp in range(1, NPAIR):
            w = wpool.tile([128, 3, D], BF16, tag="w", name=f"wqkv{hp}")
            nc.sync.dma_start(out=w, in_=wqkv_d[hp])
            wqkvs.append(w)
        wot = wopool.tile([128, 8, NPAIR, 128], BF16)
        nc.sync.dma_start(out=wot, in_=wo_d[:, :, :, :])

        state = {}   # hp -> (qT, kT, v1s)
        oTs = []

        def proj_steps(hp):
            """12 proj psum-group closures + 2 V-transpose closures."""
            qT = qkpool.tile([128, S], ADT, tag="qk", name=f"qT{hp}")
            kT = qkpool.tile([128, S], ADT, tag="qk", name=f"kT{hp}")
            vraw = vpool.tile([128, S], F32, tag="vraw", name=f"vraw{hp}")
            v1s = []
            state[hp] = (qT, kT, v1s)
            steps = []

            def pstep(sc, pj):
                with nc.named_scope(f"proj{hp}"):
                    ssl = slice(sc * SC, (sc + 1) * SC)
                    ps = psA.tile([128, SC], F32, tag="psA",
                                  name=f"pp{hp}_{sc}_{pj}")
                    for ic in range(8):
                        nc.tensor.matmul(
                            ps, wqkvs[hp][:, pj, ic * 128:(ic + 1) * 128],
                            xres[ic][sc],
                            start=(ic == 0), stop=(ic == 7))
                    if pj < 2:  # Q or K: RoPE; psum staged to bf16 on the
                        # scalar engine so the DVE ops run at 16-bit 2x rate
                        dst = qT if pj == 0 else kT
                        qr = rvpool.tile([128, SC], ADT, tag="qr",
                                         name=f"qr{hp}_{sc}_{pj}")
                        nc.scalar.copy(out=qr, in_=ps)
                        sh = rupool.tile([128, SC], ADT, tag="ru",
                                         name=f"sh{hp}_{sc}_{pj}")
                        nc.vector.stream_shuffle(out=sh, in_=qr, mask=swapmask)
                        t1 = rvpool.tile([128, SC], ADT, tag="rv",
                                         name=f"t1{hp}_{sc}_{pj}")
                        nc.vector.tensor_mul(out=t1, in0=qr, in1=ctabs[sc])
                        t2 = rupool.tile([128, SC], ADT, tag="ru",
                                         name=f"t2{hp}_{sc}_{pj}")
                        nc.vector.tensor_mul(out=t2, in0=sh, in1=stabs[sc])
                        nc.vector.tensor_add(out=dst[:, ssl], in0=t1, in1=t2)
                    else:       # V: drain to SBUF for PE transpose
                        nc.scalar.copy(out=vraw[:, ssl], in_=ps)

            v1boxes = [None, None]

            def vtstep(h, half):
                with nc.named_scope(f"vt{hp}"):
                    if v1boxes[h] is None:
                        v1boxes[h] = v1pool.tile([128, NKC, 65], ADT, tag="v1",
                                                 name=f"v1_{hp}_{h}")
                        v1s.append(v1boxes[h])
                        nc.vector.memset(v1boxes[h][:, :, 64:65], 1.0)
                    v1 = v1boxes[h]
                    pvt = psA.tile([128, 512], F32, tag="psA",
                                   name=f"pvt{hp}_{h}_{half}")
                    for j in range(8):
                        kc = half * 8 + j
                        nc.tensor.transpose(
                            pvt[:, j * 64:(j + 1) * 64],
                            vraw[h * 64:(h + 1) * 64, kc * 128:(kc + 1) * 128],
                            identT[h * 64:(h + 1) * 64, 0:64])
                    nc.vector.tensor_copy(
                        out=v1[:, half * 8:(half + 1) * 8, 0:64],
                        in_=pvt.rearrange("p (kc d) -> p kc d", d=64))

            # q/k (and their rope) first — attention of hp+1 needs qT/kT
            # and the first v1 keys earliest
            for sc in range(NSC):
                steps.append(lambda sc=sc: pstep(sc, 0))
                steps.append(lambda sc=sc: pstep(sc, 1))
                steps.append(lambda sc=sc: pstep(sc, 2))
                if sc == 1:
                    steps.append(lambda: vtstep(0, 0))
                    steps.append(lambda: vtstep(1, 0))
            steps.append(lambda: vtstep(0, 1))
            steps.append(lambda: vtstep(1, 1))
            return steps

        def attention(hp, pending):
            """attn for hp; runs `pending` closures spread over the qc loop;
            for hp==3 interleaves the output projection per q-chunk."""
            qT, kT, v1s = state[hp]
            _sc = nc.named_scope(f"attn{hp}"); _sc.__enter__()
            oT = otpool.tile([128, S], ADT, tag="oT", name=f"oT{hp}")
            oraw = orawpool.tile([128, S], F32, tag="oraw", name=f"oraw{hp}")
            hsl = (slice(0, 64), slice(64, 128))
            nsteps = len(pending)
            nkc_tot = sum(4 * q + 4 for q in range(NQC))
            kc_seen = 0
            for qc in range(NQC):
                nact = 4 * qc + 4
                qsl = slice(qc * QC, (qc + 1) * QC)
                pquads = []
                for kc in range(nact):
                    m = kc - 4 * qc
                    lo = 128 * m if m > 0 else 0
                    psq = psB.tile([128, 1024], F32, tag="big",
                                   name=f"psq{hp}_{qc}_{kc}")
                    for h in range(2):
                        nc.tensor.matmul(
                            psq[:, h * QC + lo:(h + 1) * QC],
                            kT[hsl[h], kc * KC:(kc + 1) * KC],
                            qT[hsl[h], qc * QC + lo:(qc + 1) * QC],
                            start=True, stop=(m < 0),
                            skip_group_check=True)
                    if m >= 0:
                        # staircase: add -1e9 upper-triangle on the 128-col
                        # diagonal block (keeps exp->attnV off the DVE path)
                        for h in range(2):
                            a = h * QC + lo
                            nc.tensor.matmul(
                                psq[:, a:a + 128], ident, tri,
                                start=False, stop=True,
                                skip_group_check=True)
                    pq = ppool.tile([128, 1024], ADT, tag="pT",
                                    name=f"pq{hp}_{qc}_{kc}")
                    nc.scalar.activation(
                        out=pq, in_=psq,
                        func=mybir.ActivationFunctionType.Exp, scale=0.125)
                    pquads.append(pq)
                    # spread the next head-pair's proj/vt work evenly
                    kc_seen += 1
                    want = (nsteps * kc_seen) // nkc_tot
                    while pending and nsteps - len(pending) < want:
                        pending.pop(0)()
                psos = [psO.tile([65, QC], F32, tag="psO",
                                 name=f"pso{hp}_{qc}_{h}") for h in range(2)]
                for kc in range(nact):
                    m = kc - 4 * qc
                    lo = 128 * m if m > 0 else 0
                    for h in range(2):
                        nc.tensor.matmul(
                            psos[h][:, lo:QC], v1s[h][:, kc, :],
                            pquads[kc][:, h * QC + lo:(h + 1) * QC],
                            start=(kc == 0), stop=(kc == nact - 1),
                            skip_group_check=True)
                # normalization for this q-chunk (both heads); the 1/sum
                # partition-broadcast bounces through DRAM, keeping the long
                # dependency chain off the in-order compute engines
                sq = sumpool.tile([33, QC], F32, tag="sums",
                                  name=f"sq{hp}_{qc}")
                rq = recpool.tile([33, QC], F32, tag="rec",
                                  name=f"rq{hp}_{qc}")
                drt = drpool.tile([2, QC], F32, name=f"drt{hp}_{qc}")
                recB = recpool.tile([128, QC], F32, tag="recB",
                                    name=f"recB{hp}_{qc}")
                for h in range(2):
                    nc.vector.tensor_copy(out=oraw[hsl[h], qsl],
                                          in_=psos[h][0:64, :])
                    nc.vector.tensor_copy(out=sq[32 * h:32 * h + 1, :],
                                          in_=psos[h][64:65, :])
                    nc.vector.reciprocal(
                        out=rq[32 * h:32 * h + 1, :],
                        in_=sq[32 * h:32 * h + 1, :])
                    nc.sync.dma_start(out=drt[h:h + 1, :],
                                      in_=rq[32 * h:32 * h + 1, :])
                    nc.sync.dma_start(
                        out=recB[hsl[h], :],
                        in_=drt[h:h + 1, :].to_broadcast((64, QC)))
                nc.vector.tensor_mul(out=oT[:, qsl], in0=oraw[:, qsl],
                                     in1=recB)
                if hp == NPAIR - 1:
                    with nc.named_scope("outproj"):
                        for oc in range(8):
                            ps = psA.tile([128, SC], F32, tag="psA",
                                          name=f"ops{oc}_{qc}")
                            for h2 in range(NPAIR):
                                src = oTs[h2] if h2 < len(oTs) else oT
                                nc.tensor.matmul(
                                    ps, wot[:, oc, h2, :], src[:, qsl],
                                    start=(h2 == 0), stop=(h2 == NPAIR - 1))
                            ob = obpool.tile([128, SC], F32, tag="ob",
                                             name=f"ob{oc}_{qc}")
                            nc.vector.tensor_copy(out=ob, in_=ps)
                            nc.sync.dma_start(
                                out=out_d[oc * 128:(oc + 1) * 128, qsl],
                                in_=ob)
            while pending:
                pending.pop(0)()
            _sc.__exit__(None, None, None)
            oTs.append(oT)

        # serial phases: proj/vt{hp} fully before attn{hp} (interleave via
        # INTERLEAVE=True is experimental)
        steps0 = proj_steps(0)
        for st_ in steps0[:10]:
            st_()
        for hp in range(NPAIR):
            pending = steps0[10:] if hp == 0 else []
            if hp + 1 < NPAIR:
                pending = pending + proj_steps(hp + 1)
            attention(hp, pending)

    nc.compile()
    return nc


def get_nc():
    if "nc" not in _BUILT:
        _BUILT["nc"] = _build_nc()
    return _BUILT["nc"]


def _host_prep(x, Wq, Wk, Wv, Wo, token_positions):
    bf = ml_dtypes.bfloat16
    pos = np.asarray(token_positions).astype(np.float32)
    half = DK // 2
    inv_freq = 1.0 / (10000.0 ** (np.arange(half, dtype=np.float32) * 2.0 / DK))
    ang = pos[:, None] * inv_freq[None, :]          # [S, 32]
    cos = np.cos(ang).astype(np.float32)            # [S, 32]
    sin = np.sin(ang).astype(np.float32)
    p = np.arange(128)
    j = (p % 64) // 2
    sign = np.where(p % 2 == 0, -1.0, 1.0).astype(np.float32)
    ctab = np.ascontiguousarray(cos[:, j].T).astype(bf)                 # [128, S]
    stab = np.ascontiguousarray(sin[:, j].T * sign[:, None]).astype(bf)

    kk = np.arange(KC)[:, None]
    cc = np.arange(KC)[None, :]
    tri = np.where(cc >= kk, 0.0, -1e9).astype(bf)   # [KC, KC] additive mask
    ident = np.eye(128, dtype=bf)
    identT = np.vstack([np.eye(64, dtype=np.float32)] * 2)

    in_maps = []
    for c in range(NCORES):
        b, hf = divmod(c, 2)
        m = {}
        # [2, 128, 8, S/2]: xpk[half, p, ic, s] = xT[ic*128+p, half*S/2+s]
        m["xT"] = np.ascontiguousarray(
            x[b].T.reshape(8, 128, 2, S // 2).transpose(2, 1, 0, 3)
        ).astype(bf)
        # wqkv[hp, p, pj, ic*128+j] = Wpj[hf*512 + hp*128 + j, ic*128 + p]
        wqkv = np.empty((NPAIR, 128, 3, D), dtype=bf)
        for pj, W in enumerate((Wq, Wk, Wv)):
            Ws = W[hf * 512:(hf + 1) * 512, :]       # [512 out, 1024 in]
            A = Ws.reshape(NPAIR, 128, 8, 128)       # [hp, jout, ic, pin]
            wqkv[:, :, pj, :] = A.transpose(0, 3, 2, 1).reshape(NPAIR, 128, D)
        m["wqkv"] = wqkv
        # wopk[p, oc, hp, j] = WoT[hp*128+p, oc*128+j];  WoT = Wo[:, cols].T
        WoT = Wo[:, hf * 512:(hf + 1) * 512].T       # [512, 1024]
        Bm = WoT.reshape(NPAIR, 128, 8, 128)         # [hp, p, oc, j]
        m["wopk"] = np.ascontiguousarray(
            Bm.transpose(1, 2, 0, 3)).astype(bf)     # [128, 8, NPAIR, 128]
        m["ctab"] = ctab
        m["stab"] = stab
        m["masks"] = tri
        m["ident"] = ident
        m["identT"] = identT
        in_maps.append(m)
    return in_maps


def run(inputs, trace=False, **kw):
    in_maps = _host_prep(**{k: np.asarray(v) for k, v in inputs.items()})
    nc = get_nc()
    res = run_bass_kernel_spmd(nc, in_maps, list(range(NCORES)), trace=trace, **kw)
    outs = [res.results[c]["outP"] for c in range(NCORES)]
    out = np.stack([(outs[2 * b] + outs[2 * b + 1]).T for b in range(B)])
    return out.astype(np.float32), res


def kernel(**inputs):
    out, _ = run(inputs, trace=False)
    return out
